# revision 23
# baseline (speedup 1.0000x reference)
"""A2C2f Trainium2 kernel: 8-core SPMD via bass/tile.

Sharding: core = b*4 + s.  b = batch (2), s = sequence-slice (4).
The reference's area reshape on (B, C, N) row-major arrays maps
  (b, c, n) -> group g = b*4 + c//128 (qk) / c//64 (v), head h, d = 4*uu + qhat,
  sequence pos j = n % 1024   (qhat = n//1024 spatial quarter, uu low channel bits).
Each core owns sequence slice j in [256*s, 256*s+256) for ALL groups/heads of its
batch => queries local, keys/values AllGathered (2 collectives per block).
Attention: S^T = k^T q row-tiled 4 heads/matmul (K=32); exp on ACT (scale folded);
L = ones-reduce col-tiled; out2 col-tiled (M=32); normalize = DVE reciprocal+mul.
Depthwise 5x5 PE conv: v4 gathered in channel layout, own rows +-2 selected with a
partition_id-derived register (dynamic DMA), 25 in-place DVE FMAs.
"""
import os, sys, types
sys.path.insert(0, '/opt/trn_rl_repo')
import numpy as np
import ml_dtypes

if "antenv.axon_hooks" not in sys.modules:
    _m = types.ModuleType("antenv.axon_hooks")
    _h = [None]
    _m.set_axon_ntff_profile_hook = lambda h: _h.__setitem__(0, h)
    _m.get_axon_ntff_profile_hook = lambda: _h[0]
    sys.modules["antenv.axon_hooks"] = _m
    try:
        import antenv
        antenv.axon_hooks = _m
        from trn_agent_boot.trn_boot import _ntff_profile_via_ctypes
        _m.set_axon_ntff_profile_hook(_ntff_profile_via_ctypes('/opt/axon/libaxon_pjrt.so'))
    except Exception:
        pass

import concourse.bass as bass
import concourse.bacc as bacc
import concourse.mybir as mybir
import concourse.tile as tile
import concourse.bass_utils as bass_utils
from concourse.bass_utils import run_bass_kernel_spmd

bass_utils.upload_artifacts = lambda tmpdir: "local://skipped"

dt = mybir.dt
AF = mybir.ActivationFunctionType
ALU = mybir.AluOpType
bf16 = ml_dtypes.bfloat16

NC = 8
B, C1, C2, H, W = 2, 512, 512, 64, 64
CH = 256
NB = 4
N = H * W
J = 1024
II = 256
SCALE = 32 ** -0.5
STAGE = int(os.environ.get("A2_STAGE", "9"))
ATT = int(os.environ.get("A2_ATT", "9"))
PSV = int(os.environ.get("A2_PSV", "1"))
NBB = int(os.environ.get("A2_NB", "4"))

_cache = {}


def _build():
    nc = bacc.Bacc("TRN2", target_bir_lowering=False, debug=False, num_devices=NC)

    x_bf = nc.declare_dram_parameter("x_bf", [C1, 1024], dt.bfloat16, isOutput=False)
    P = {}
    def par(name, shape, d=dt.bfloat16):
        P[name] = nc.declare_dram_parameter(name, shape, d, isOutput=False)
    par("cv1_wt", [C1, CH]); par("cv1_b", [CH, 1], dt.float32)
    par("cv2_wt", [3 * CH, C2]); par("cv2_b", [C2, 1], dt.float32)
    par("gamma", [C2, 1], dt.float32)
    for t in range(NB):
        par(f"b{t}_qk_wt", [CH, 2 * CH])
        par(f"b{t}_qb", [CH, 1], dt.float32); par(f"b{t}_kb", [CH, 1], dt.float32)
        par(f"b{t}_v_wt", [CH, CH]); par(f"b{t}_v_b", [CH, 1], dt.float32)
        par(f"b{t}_vb_d", [8 * 128, 1], dt.float32)
        par(f"b{t}_pe_w", [CH, 25], dt.float32); par(f"b{t}_pe_b", [CH, 1], dt.float32)
        par(f"b{t}_proj_wt", [CH, CH]); par(f"b{t}_proj_b", [CH, 1], dt.float32)
        par(f"b{t}_mlp1_wt", [CH, 2 * CH]); par(f"b{t}_mlp1_b", [2 * CH, 1], dt.float32)
        par(f"b{t}_mlp2_wt", [2 * CH, CH]); par(f"b{t}_mlp2_b", [CH, 1], dt.float32)
    out_ext = nc.declare_dram_parameter("out", [C2, 1024], dt.float32, isOutput=True)

    qdram = [nc.dram_tensor(f"qdram{t}", [J, II], dt.bfloat16) for t in range(NB)]
    odram = [nc.dram_tensor(f"odram{t}", [CH, 1024], dt.bfloat16) for t in range(NB)]
    agin_k = [nc.dram_tensor(f"agin_k{t}", [J, II], dt.bfloat16) for t in range(NB)]
    agout_k = [nc.dram_tensor(f"agout_k{t}", [4 * J, II], dt.bfloat16) for t in range(NB)]
    agin_va = [nc.dram_tensor(f"agin_va{t}", [CH, 1024], dt.bfloat16) for t in range(NB)]
    agout_va = [nc.dram_tensor(f"agout_va{t}", [4 * CH, 1024], dt.bfloat16) for t in range(NB)]
    agin_vb = [nc.dram_tensor(f"agin_vb{t}", [CH, 1024], dt.bfloat16) for t in range(NB)]
    agout_vb = [nc.dram_tensor(f"agout_vb{t}", [4 * CH, 1024], dt.bfloat16) for t in range(NB)]
    dummy_in = nc.dram_tensor("dummy_in", [1, 64], dt.float32)
    dummy_out = nc.dram_tensor("dummy_out", [8, 64], dt.float32, addr_space="Shared")

    RG = [[0, 1, 2, 3], [4, 5, 6, 7]]
    RG8 = [list(range(8))]
    Pool = mybir.EngineType.Pool

    with tile.TileContext(nc) as tc:
        with (
            tc.tile_pool(name="wp", bufs=1) as wp,
            tc.tile_pool(name="sp", bufs=2) as sp,
            tc.tile_pool(name="ps", bufs=2, space="PSUM") as ps_pool,
            tc.tile_pool(name="ps1", bufs=1, space="PSUM") as ps1_pool,
        ):
            # dummy collective to absorb ncfw init (overlaps weight loads)
            zt = wp.tile([1, 64], dt.float32, name="zt")
            nc.vector.memset(zt[:], 0.0)
            nc.gpsimd.dma_start(dummy_in[:], zt[:])
            nc.gpsimd.collective_compute(
                "AllGather", ALU.bypass, replica_groups=RG8,
                ins=[dummy_in[:].opt()], outs=[dummy_out[:].opt()])

            # per-quarter dynamic row offsets: 16*qh + 4*(pid % 4)
            pid = nc.partition_id(engines=[Pool])
            regq = []
            for qh in range(4):
                r = nc.alloc_registers(f"regq{qh}", engines=[Pool])
                nc.regs_alu(r, pid, 4, op=ALU.mod)
                nc.regs_alu(r, r, 4, op=ALU.mult)
                nc.regs_alu(r, r, 16 * qh, op=ALU.add)
                regq.append(nc.snap(r, donate=True, min_val=16 * qh, max_val=16 * qh + 12))

            def wload(name, rows, cols, d=dt.bfloat16):
                ts = []
                for i in range((rows + 127) // 128):
                    t_ = wp.tile([min(128, rows - 128 * i), cols], d, name=f"w_{name}_{i}")
                    nc.sync.dma_start(t_[:], P[name][128 * i:min(rows, 128 * i + 128), :])
                    ts.append(t_)
                return ts

            cv1_wt = wload("cv1_wt", C1, CH)
            cv2_wt = wload("cv2_wt", 3 * CH, C2)
            cv1_b = wload("cv1_b", CH, 1, dt.float32)
            cv2_b = wload("cv2_b", C2, 1, dt.float32)
            gamma = wload("gamma", C2, 1, dt.float32)
            WB = []
            for t in range(NB):
                d_ = {}
                for k, rows, cols, dd in [
                    ("qk_wt", CH, 2 * CH, dt.bfloat16), ("v_wt", CH, CH, dt.bfloat16),
                    ("proj_wt", CH, CH, dt.bfloat16), ("mlp1_wt", CH, 2 * CH, dt.bfloat16),
                    ("mlp2_wt", 2 * CH, CH, dt.bfloat16),
                    ("qb", CH, 1, dt.float32), ("kb", CH, 1, dt.float32),
                    ("v_b", CH, 1, dt.float32), ("vb_d", 8 * 128, 1, dt.float32),
                    ("pe_w", CH, 25, dt.float32), ("pe_b", CH, 1, dt.float32),
                    ("proj_b", CH, 1, dt.float32), ("mlp1_b", 2 * CH, 1, dt.float32),
                    ("mlp2_b", CH, 1, dt.float32),
                ]:
                    d_[k] = wload(f"b{t}_{k}", rows, cols, dd)
                WB.append(d_)

            ones32 = wp.tile([128, 32], dt.bfloat16, name="ones32")
            nc.vector.memset(ones32[:], 1.0)

            ybf = [[wp.tile([128, 1024], dt.bfloat16, name=f"ybf{t}_{m}") for m in range(2)]
                   for t in range(NB + 1)]
            yf32 = [wp.tile([128, 1024], dt.float32, name=f"yf32_{m}") for m in range(2)]
            v4fp = [wp.tile([128, 68 * 64], dt.bfloat16, name=f"v4fp{m}") for m in range(2)]
            for m in range(2):
                nc.vector.memset(v4fp[m][:, 0:128], 0.0)
                nc.vector.memset(v4fp[m][:, 66 * 64:68 * 64], 0.0)

            xbf = []
            for c in range(4):
                xb = sp.tile([128, 1024], dt.bfloat16, tag="hbf", bufs=4, name="xbf")
                nc.sync.dma_start(xb[:], x_bf[128 * c:128 * c + 128, :])
                xbf.append(xb)

            # cv1
            for m in range(2):
                for n_ in range(2):
                    ps = ps_pool.tile([128, 512], dt.float32, tag="ps_s0", bufs=2, name="ps_cv1")
                    for cc in range(4):
                        nc.tensor.matmul(ps[:], cv1_wt[cc][:, 128 * m:128 * m + 128],
                                         xbf[cc][:, 512 * n_:512 * n_ + 512],
                                         start=(cc == 0), stop=(cc == 3))
                    nc.scalar.activation(yf32[m][:, 512 * n_:512 * n_ + 512], ps[:],
                                         AF.Silu, bias=cv1_b[m][:])
                nc.vector.tensor_copy(ybf[0][m][:], yf32[m][:])

            # ================= blocks =================
            for t in range(NBB):
                Wt = WB[t]
                ycur = ybf[t]
                if STAGE == 0:
                    for m in range(2):
                        nc.vector.tensor_copy(ybf[t + 1][m][:], yf32[m][:])
                    continue

                # q/k convs -> staging tiles (cg-pair layout)
                if True:
                    pass
                st_q = [[None] * 2 for _ in range(4)]
                st_k = [[None] * 2 for _ in range(4)]
                for qh in (range(4) if STAGE >= 1 else []):
                    for pg in range(2):
                        psq = ps_pool.tile([128, 256], dt.float32, tag="ps_s0", bufs=2, name="psq")
                        psk = ps_pool.tile([128, 256], dt.float32, tag="ps_s1", bufs=2, name="psk")
                        for cc in range(2):
                            rhs = ycur[cc][:, 256 * qh:256 * qh + 256]
                            for hf in range(2):
                                cg = 2 * pg + hf
                                nc.tensor.matmul(
                                    psq[64 * hf:64 * hf + 64, :],
                                    Wt["qk_wt"][cc][:, 128 * cg:128 * cg + 64], rhs,
                                    start=(cc == 0), stop=(cc == 1), tile_position=(0, 64 * hf))
                                nc.tensor.matmul(
                                    psk[64 * hf:64 * hf + 64, :],
                                    Wt["qk_wt"][cc][:, 128 * cg + 64:128 * cg + 128], rhs,
                                    start=(cc == 0), stop=(cc == 1), tile_position=(0, 64 * hf))
                        sq = sp.tile([128, 256], dt.bfloat16, tag="stg", bufs=6, name="sq")
                        sk = sp.tile([128, 256], dt.bfloat16, tag="stg", bufs=6, name="sk")
                        nc.vector.tensor_scalar(sq[:], psq[:], Wt["qb"][pg][:], None, op0=ALU.add)
                        nc.vector.tensor_scalar(sk[:], psk[:], Wt["kb"][pg][:], None, op0=ALU.add)
                        st_q[qh][pg] = sq; st_k[qh][pg] = sk

                # scatter q/k into d-layout dram: row r = g*256 + h*32 + uu*4 + qh
                for qh in (range(4) if STAGE >= 1 else []):
                    for pg in range(2):
                        for dst_dram, st in ((qdram[t], st_q), (agin_k[t], st_k)):
                            d4 = dst_dram.rearrange("(g h u q) j -> g h u q j", g=4, h=8, u=8)
                            nc.gpsimd.dma_start(
                                d4[2 * pg:2 * pg + 2, :, :, qh, :], st[qh][pg][:])

                if STAGE >= 1:
                    nc.gpsimd.collective_compute(
                        "AllGather", ALU.bypass, replica_groups=RG,
                        ins=[agin_k[t][:].opt()], outs=[agout_k[t][:].opt()])

                # vT conv (j on partitions); staging (jj, c')
                svt = [sp.tile([128, 1024], dt.bfloat16, tag=f"svt{jt}", name="svt")
                       for jt in range(2)]
                for qh in (range(4) if STAGE >= 1 else []):
                    for jt in range(2):
                        psv = ps_pool.tile([128, 256], dt.float32, tag="ps_s0", bufs=2, name="psv")
                        for cc in range(2):
                            nc.tensor.matmul(
                                psv[:],
                                ycur[cc][:, 256 * qh + 128 * jt:256 * qh + 128 * jt + 128],
                                Wt["v_wt"][cc][:], start=(cc == 0), stop=(cc == 1))
                        for gg in range(4):
                            dstv = svt[jt].rearrange("p (g h u q) -> p g h u q",
                                                     g=4, h=8, u=8)[:, gg, :, :, qh]
                            nc.vector.tensor_copy(
                                dstv, psv.rearrange("p (g h u) -> p g h u",
                                                    g=4, h=8)[:, gg, :, :])
                if STAGE >= 1:
                    nc.gpsimd.dma_start(agin_va[t][0:128, :], svt[0][:])
                    nc.gpsimd.dma_start(agin_va[t][128:256, :], svt[1][:])
                    nc.gpsimd.collective_compute(
                        "AllGather", ALU.bypass, replica_groups=RG,
                        ins=[agin_va[t][:].opt()], outs=[agout_va[t][:].opt()])

                # v4 conv in channel layout (own positions)
                for m in (range(2) if STAGE >= 1 else []):
                    for n_ in range(2):
                        psc = ps_pool.tile([128, 512], dt.float32, tag="ps_s1", bufs=2, name="psc")
                        for cc in range(2):
                            nc.tensor.matmul(psc[:], Wt["v_wt"][cc][:, 128 * m:128 * m + 128],
                                             ycur[cc][:, 512 * n_:512 * n_ + 512],
                                             start=(cc == 0), stop=(cc == 1))
                        sv4 = sp.tile([128, 512], dt.bfloat16, tag="sv4", name="sv4")
                        nc.vector.tensor_scalar(sv4[:], psc[:], Wt["v_b"][m][:], None, op0=ALU.add)
                        nc.gpsimd.dma_start(
                            agin_vb[t][128 * m:128 * m + 128,
                                       512 * n_:512 * n_ + 512], sv4[:])

                if STAGE >= 1:
                    nc.gpsimd.collective_compute(
                        "AllGather", ALU.bypass, replica_groups=RG,
                        ins=[agin_vb[t][:].opt()], outs=[agout_vb[t][:].opt()])

                # read back K, Q, vT
                kt_, qt_, vt_ = [], [], []
                agk4 = agout_k[t].rearrange("(r p) j -> r p j", r=4)
                for tt in (range(8) if STAGE >= 2 else []):
                    ktile = sp.tile([128, 1024], dt.bfloat16, tag=f"kt{tt}", bufs=1, name="ktile")
                    nc.scalar.dma_start(
                        ktile[:], agk4[:, 128 * tt:128 * tt + 128, :].transpose([1, 0, 2]))
                    kt_.append(ktile)
                    qtile = sp.tile([128, 256], dt.bfloat16, tag=f"qt{tt}", bufs=1, name="qtile")
                    nc.sync.dma_start(qtile[:], qdram[t][128 * tt:128 * tt + 128, :])
                    qt_.append(qtile)
                    vtile = sp.tile([128, 1024], dt.bfloat16, tag=f"vt{tt}", bufs=1, name="vtile")
                    nc.scalar.dma_start(vtile[:], agout_va[t][128 * tt:128 * tt + 128, :])
                    vt_.append(vtile)

                # v4full (channel layout, all spatial rows) into padded buffer
                agv4 = agout_vb[t].rearrange("(r c) n -> r c n", r=4)
                for m in (range(2) if STAGE >= 2 else []):
                    for qh in range(4):
                        src = agv4[:, 128 * m:128 * m + 128,
                                   256 * qh:256 * qh + 256]
                        src = src.rearrange("r c (w x) -> r c w x", x=64).transpose([1, 0, 2, 3])
                        nc.scalar.dma_start(
                            v4fp[m][:, 64 * (16 * qh + 2):64 * (16 * qh + 2) + 1024], src)

                # dynamic gather of own rows +-2 per quarter
                v4e = [sp.tile([128, 2048], dt.bfloat16, tag=f"v4e{m}", bufs=1, name="v4e")
                       for m in range(2)]
                for m in (range(2) if STAGE >= 3 else []):
                    v3 = v4fp[m].rearrange("p (r x) -> p r x", x=64)
                    for qh in range(4):
                        nc.gpsimd.dma_start(v4e[m][:, 512 * qh:512 * qh + 512],
                                            v3[:, bass.ds(regq[qh], 8), :])

                # depthwise 5x5
                pp = [sp.tile([128, 1024], dt.bfloat16, tag=f"pp{m}", bufs=1, name="pp")
                      for m in range(2)]
                if STAGE < 3:
                    for m in range(2):
                        nc.vector.memset(pp[m][:], 0.0)
                for m in (range(2) if STAGE >= 3 else []):
                    o4 = pp[m].rearrange("p (q r x) -> p q r x", q=4, r=4)
                    i4 = v4e[m].rearrange("p (q r x) -> p q r x", q=4, r=8)
                    ctap = 12
                    for qh in range(4):
                        nc.vector.tensor_scalar(o4[:, qh, :, :], i4[:, qh, 2:6, :],
                                                Wt["pe_w"][m][:, ctap:ctap + 1],
                                                Wt["pe_b"][m][:], op0=ALU.mult, op1=ALU.add)
                    for dy in range(5):
                        for dx in range(5):
                            if dy == 2 and dx == 2:
                                continue
                            tap = dy * 5 + dx
                            a = max(0, 2 - dx); b = min(64, 66 - dx)
                            for qh in range(4):
                                nc.vector.scalar_tensor_tensor(
                                    o4[:, qh, :, a:b],
                                    i4[:, qh, dy:dy + 4, a - 2 + dx:b - 2 + dx],
                                    Wt["pe_w"][m][:, tap:tap + 1],
                                    o4[:, qh, :, a:b], op0=ALU.mult, op1=ALU.add)

                # attention per (g, hh)
                for g in (range(4) if STAGE >= 4 else []):
                    for hh in range(2):
                        tt = 2 * g + hh
                        ps_o = ps_l = None
                        if ATT >= 2:
                            ps_o = ps1_pool.tile([128, 256], dt.float32, tag="ps_o", name="ps_o")
                            ps_l = ps1_pool.tile([128, 256], dt.float32, tag="ps_l", name="ps_l")
                        for jp in range(4):
                            us2 = []
                            for h2 in range(4):
                                ps_s1 = ps_pool.tile([128, 512], dt.float32,
                                                     tag=f"ps_s{h2}", bufs=(2 if h2 < 2 else 1), name="ps_s1")
                                for sub in range(2):
                                    jt = 2 * jp + sub
                                    nc.tensor.matmul(
                                        ps_s1[:, 256 * sub:256 * sub + 256],
                                        kt_[tt][32 * h2:32 * h2 + 32, 128 * jt:128 * jt + 128],
                                        qt_[tt][32 * h2:32 * h2 + 32, :],
                                        start=True, stop=True, tile_position=(32 * h2, 0))
                                u2 = sp.tile([128, 512], dt.bfloat16, tag=f"u{jp}h{h2}",
                                             bufs=1, name="u2")
                                nc.scalar.activation(u2[:], ps_s1[:], AF.Exp, scale=SCALE)
                                us2.append(u2)
                            for h2 in range(4):
                                c0 = 256 * g + 32 * (4 * hh + h2)
                                for sub in range(2):
                                    jt = 2 * jp + sub
                                    nc.tensor.matmul(
                                        ps_o[32 * h2:32 * h2 + 32, :],
                                        vt_[jt][:, c0:c0 + 32],
                                        us2[h2][:, 256 * sub:256 * sub + 256],
                                        start=(jp == 0 and sub == 0),
                                        stop=(jp == 3 and sub == 1), tile_position=(0, 32 * h2))
                                    nc.tensor.matmul(
                                        ps_l[32 * h2:32 * h2 + 32, :],
                                        ones32[:],
                                        us2[h2][:, 256 * sub:256 * sub + 256],
                                        start=(jp == 0 and sub == 0),
                                        stop=(jp == 3 and sub == 1), tile_position=(0, 32 * h2))
                        if ATT == 1:
                            continue
                        if ATT == 2:
                            o_dbg = sp.tile([128, 256], dt.float32, tag="o_f", name="o_dbg")
                            nc.vector.tensor_copy(o_dbg[:], ps_o[:])
                            nc.gpsimd.dma_start(odram[t][32 * tt:32 * tt + 1, 0:4], o_dbg[0:1, 0:4])
                            continue
                        r_sb = sp.tile([128, 256], dt.float32, tag="r_sb", name="r_sb")
                        nc.vector.reciprocal(r_sb[:], ps_l[:])
                        o_f = sp.tile([128, 256], dt.float32, tag="o_f", name="o_f")
                        nc.vector.tensor_tensor(o_f[:], ps_o[:], r_sb[:], op=ALU.mult)
                        o_st = sp.tile([128, 256], dt.bfloat16, tag="o_st", name="o_st")
                        nc.vector.tensor_scalar(o_st[:], o_f[:], Wt["vb_d"][tt][:], None,
                                                op0=ALU.add)
                        if ATT == 3:
                            nc.gpsimd.dma_start(odram[t][32 * tt:32 * tt + 1, 0:4], o_st[0:1, 0:4])
                        else:
                            for h2 in range(4):
                                dsto = odram[t][32 * tt + 8 * h2:32 * tt + 8 * h2 + 8, :] \
                                    .rearrange("u (q j) -> u q j", q=4)
                                nc.gpsimd.dma_start(dsto, o_st[32 * h2:32 * h2 + 32, :])

                # o + pp -> proj -> residual
                oc, ao = [], []
                for m in range(2):
                    oct = sp.tile([128, 1024], dt.bfloat16, tag=f"oc{m}", bufs=1, name="oct")
                    if STAGE >= 4:
                        nc.sync.dma_start(oct[:], odram[t][128 * m:128 * m + 128, :])
                    else:
                        nc.vector.memset(oct[:], 0.0)
                    oc.append(oct)
                for m in range(2):
                    aot = sp.tile([128, 1024], dt.bfloat16, tag=f"ao{m}", bufs=1, name="aot")
                    nc.vector.tensor_tensor(aot[:], oc[m][:], pp[m][:], op=ALU.add)
                    ao.append(aot)
                ymid = []
                for m in range(2):
                    ym = sp.tile([128, 1024], dt.bfloat16, tag=f"ym{m}", bufs=1, name="ym")
                    for n_ in range(2):
                        psp = ps_pool.tile([128, 512], dt.float32, tag="ps_s0", bufs=2, name="psp")
                        for cc in range(2):
                            nc.tensor.matmul(psp[:], Wt["proj_wt"][cc][:, 128 * m:128 * m + 128],
                                             ao[cc][:, 512 * n_:512 * n_ + 512],
                                             start=(cc == 0), stop=(cc == 1))
                        nc.vector.scalar_tensor_tensor(
                            yf32[m][:, 512 * n_:512 * n_ + 512], psp[:], Wt["proj_b"][m][:],
                            yf32[m][:, 512 * n_:512 * n_ + 512], op0=ALU.add, op1=ALU.add)
                    nc.vector.tensor_copy(ym[:], yf32[m][:])
                    ymid.append(ym)

                # mlp
                hbf = []
                for mt in range(4):
                    hb = sp.tile([128, 1024], dt.bfloat16, tag="hbf", bufs=4, name="hbf")
                    for n_ in range(2):
                        ps1_ = ps_pool.tile([128, 512], dt.float32, tag="ps_s1", bufs=2, name="ps_mlp1")
                        for cc in range(2):
                            nc.tensor.matmul(ps1_[:], Wt["mlp1_wt"][cc][:, 128 * mt:128 * mt + 128],
                                             ymid[cc][:, 512 * n_:512 * n_ + 512],
                                             start=(cc == 0), stop=(cc == 1))
                        nc.scalar.activation(hb[:, 512 * n_:512 * n_ + 512], ps1_[:],
                                             AF.Silu, bias=Wt["mlp1_b"][mt][:])
                    hbf.append(hb)
                for m in range(2):
                    for n_ in range(2):
                        ps2_ = ps_pool.tile([128, 512], dt.float32, tag="ps_s0", bufs=2, name="ps_mlp2")
                        for cc in range(4):
                            nc.tensor.matmul(ps2_[:], Wt["mlp2_wt"][cc][:, 128 * m:128 * m + 128],
                                             hbf[cc][:, 512 * n_:512 * n_ + 512],
                                             start=(cc == 0), stop=(cc == 3))
                        nc.vector.scalar_tensor_tensor(
                            yf32[m][:, 512 * n_:512 * n_ + 512], ps2_[:], Wt["mlp2_b"][m][:],
                            yf32[m][:, 512 * n_:512 * n_ + 512], op0=ALU.add, op1=ALU.add)
                    nc.vector.tensor_copy(ybf[t + 1][m][:], yf32[m][:])

            # cv2 + delta out (final x-residual added host-side)
            ycat = [ybf[0][0], ybf[0][1], ybf[min(2, NBB)][0], ybf[min(2, NBB)][1], ybf[NBB][0], ybf[NBB][1]]
            for m in range(4):
                for n_ in range(2):
                    psf = ps_pool.tile([128, 512], dt.float32, tag="ps_s1", bufs=2, name="ps_cv2")
                    for cc in range(6):
                        nc.tensor.matmul(psf[:], cv2_wt[cc][:, 128 * m:128 * m + 128],
                                         ycat[cc][:, 512 * n_:512 * n_ + 512],
                                         start=(cc == 0), stop=(cc == 5))
                    os_ = sp.tile([128, 512], dt.float32, tag="os", name="os_")
                    nc.scalar.activation(os_[:], psf[:], AF.Silu, bias=cv2_b[m][:])
                    dl = sp.tile([128, 512], dt.float32, tag="dl", name="dl")
                    nc.vector.tensor_scalar(dl[:], os_[:], gamma[m][:], None, op0=ALU.mult)
                    nc.sync.dma_start(out_ext[128 * m:128 * m + 128, 512 * n_:512 * n_ + 512],
                                      dl[:])

    nc.compile()
    return nc


def _prep_inputs(x, params):
    xf = np.asarray(x, np.float32).reshape(B, C1, N)
    f32 = lambda a: np.ascontiguousarray(np.asarray(a, np.float32))
    bfc = lambda a: np.ascontiguousarray(np.asarray(a, np.float32).astype(bf16))
    col = lambda a: f32(a).reshape(-1, 1)

    shared = {
        "cv1_wt": bfc(np.asarray(params["cv1_w"]).T), "cv1_b": col(params["cv1_b"]),
        "cv2_wt": bfc(np.asarray(params["cv2_w"]).T), "cv2_b": col(params["cv2_b"]),
        "gamma": col(params["gamma"]),
    }
    blocks = [bp for stack in params["blocks"] for bp in stack]
    for t, bp in enumerate(blocks):
        qkw = np.asarray(bp["qk_w"], np.float32)
        shared[f"b{t}_qk_wt"] = bfc(qkw.T)
        qkb = np.asarray(bp["qk_b"], np.float32)
        qb = np.zeros((CH,), np.float32); kb = np.zeros((CH,), np.float32)
        for pg in range(2):
            for half in range(2):
                cg = 2 * pg + half
                qb[128 * pg + 64 * half:128 * pg + 64 * half + 64] = \
                    qkb[128 * cg:128 * cg + 64]
                kb[128 * pg + 64 * half:128 * pg + 64 * half + 64] = \
                    qkb[128 * cg + 64:128 * cg + 128]
        shared[f"b{t}_qb"] = qb.reshape(-1, 1); shared[f"b{t}_kb"] = kb.reshape(-1, 1)
        shared[f"b{t}_v_wt"] = bfc(np.asarray(bp["v_w"]).T)
        vb = np.asarray(bp["v_b"], np.float32)
        shared[f"b{t}_v_b"] = vb.reshape(-1, 1)
        vbd = np.zeros((8, 128), np.float32)
        for g in range(4):
            for hh in range(2):
                for h2 in range(4):
                    for uu in range(8):
                        for qh in range(4):
                            vbd[2 * g + hh, 32 * h2 + 4 * uu + qh] = \
                                vb[64 * g + 8 * (4 * hh + h2) + uu]
        shared[f"b{t}_vb_d"] = vbd.reshape(-1, 1)
        shared[f"b{t}_pe_w"] = f32(np.asarray(bp["pe_w"]).reshape(CH, 25))
        shared[f"b{t}_pe_b"] = col(bp["pe_b"])
        shared[f"b{t}_proj_wt"] = bfc(np.asarray(bp["proj_w"]).T)
        shared[f"b{t}_proj_b"] = col(bp["proj_b"])
        shared[f"b{t}_mlp1_wt"] = bfc(np.asarray(bp["mlp1_w"]).T)
        shared[f"b{t}_mlp1_b"] = col(bp["mlp1_b"])
        shared[f"b{t}_mlp2_wt"] = bfc(np.asarray(bp["mlp2_w"]).T)
        shared[f"b{t}_mlp2_b"] = col(bp["mlp2_b"])

    in_maps = []
    for core in range(NC):
        b, s = core // 4, core % 4
        cols = (np.arange(4)[:, None] * 1024 + 256 * s + np.arange(256)[None, :]).reshape(-1)
        m = dict(shared)
        m["x_bf"] = np.ascontiguousarray(xf[b][:, cols].astype(bf16))
        in_maps.append(m)
    return in_maps, xf


def _run(x, params, trace=False):
    if "nc" not in _cache:
        _cache["nc"] = _build()
    nc = _cache["nc"]
    in_maps, xf = _prep_inputs(x, params)
    res = run_bass_kernel_spmd(nc, in_maps, core_ids=list(range(NC)), trace=trace)
    out = np.empty((B, C2, N), np.float32)
    for core in range(NC):
        b, s = core // 4, core % 4
        cols = (np.arange(4)[:, None] * 1024 + 256 * s + np.arange(256)[None, :]).reshape(-1)
        out[b][:, cols] = xf[b][:, cols] + res.results[core]["out"]
    return out.reshape(B, C2, H, W), res


def kernel(x, params):
    out, _ = _run(x, params, trace=False)
    return out


# revision 24
# speedup vs baseline: 1.2297x; 1.2297x over previous
"""A2C2f Trainium2 kernel: 8-core SPMD via bass/tile.

Sharding: core = b*4 + s.  b = batch (2), s = sequence-slice (4).
The reference's area reshape on (B, C, N) row-major arrays maps
  (b, c, n) -> group g = b*4 + c//128 (qk) / c//64 (v), head h, d = 4*uu + qhat,
  sequence pos j = n % 1024   (qhat = n//1024 spatial quarter, uu low channel bits).
Each core owns sequence slice j in [256*s, 256*s+256) for ALL groups/heads of its
batch => queries local, keys/values AllGathered (2 collectives per block).
Attention: S^T = k^T q row-tiled 4 heads/matmul (K=32); exp on ACT (scale folded);
L = ones-reduce col-tiled; out2 col-tiled (M=32); normalize = DVE reciprocal+mul.
Depthwise 5x5 PE conv: v4 gathered in channel layout, own rows +-2 selected with a
partition_id-derived register (dynamic DMA), 25 in-place DVE FMAs.
"""
import os, sys, types
sys.path.insert(0, '/opt/trn_rl_repo')
import numpy as np
import ml_dtypes

if "antenv.axon_hooks" not in sys.modules:
    _m = types.ModuleType("antenv.axon_hooks")
    _h = [None]
    _m.set_axon_ntff_profile_hook = lambda h: _h.__setitem__(0, h)
    _m.get_axon_ntff_profile_hook = lambda: _h[0]
    sys.modules["antenv.axon_hooks"] = _m
    try:
        import antenv
        antenv.axon_hooks = _m
        from trn_agent_boot.trn_boot import _ntff_profile_via_ctypes
        _m.set_axon_ntff_profile_hook(_ntff_profile_via_ctypes('/opt/axon/libaxon_pjrt.so'))
    except Exception:
        pass

import concourse.bass as bass
import concourse.bacc as bacc
import concourse.mybir as mybir
import concourse.tile as tile
import concourse.bass_utils as bass_utils
from concourse.bass_utils import run_bass_kernel_spmd

bass_utils.upload_artifacts = lambda tmpdir: "local://skipped"

dt = mybir.dt
AF = mybir.ActivationFunctionType
ALU = mybir.AluOpType
bf16 = ml_dtypes.bfloat16

NC = 8
B, C1, C2, H, W = 2, 512, 512, 64, 64
CH = 256
NB = 4
N = H * W
J = 1024
II = 256
SCALE = 32 ** -0.5
STAGE = int(os.environ.get("A2_STAGE", "9"))
ATT = int(os.environ.get("A2_ATT", "9"))
PSV = int(os.environ.get("A2_PSV", "1"))
NBB = int(os.environ.get("A2_NB", "4"))

_cache = {}


def _build():
    nc = bacc.Bacc("TRN2", target_bir_lowering=False, debug=False, num_devices=NC)

    x_bf = nc.declare_dram_parameter("x_bf", [C1, 1024], dt.bfloat16, isOutput=False)
    P = {}
    def par(name, shape, d=dt.bfloat16):
        P[name] = nc.declare_dram_parameter(name, shape, d, isOutput=False)
    par("cv1_wt", [C1, CH]); par("cv1_b", [CH, 1], dt.float32)
    par("cv2_wt", [3 * CH, C2]); par("cv2_b", [C2, 1], dt.float32)
    par("gamma", [C2, 1], dt.float32)
    for t in range(NB):
        par(f"b{t}_qk_wt", [CH, 2 * CH])
        par(f"b{t}_qb", [CH, 1], dt.float32); par(f"b{t}_kb", [CH, 1], dt.float32)
        par(f"b{t}_v_wt", [CH, CH]); par(f"b{t}_v_b", [CH, 1], dt.float32)
        par(f"b{t}_vb_d", [8 * 128, 1], dt.float32)
        par(f"b{t}_pe_w", [CH, 25], dt.float32); par(f"b{t}_pe_b", [CH, 1], dt.float32)
        par(f"b{t}_proj_wt", [CH, CH]); par(f"b{t}_proj_b", [CH, 1], dt.float32)
        par(f"b{t}_mlp1_wt", [CH, 2 * CH]); par(f"b{t}_mlp1_b", [2 * CH, 1], dt.float32)
        par(f"b{t}_mlp2_wt", [2 * CH, CH]); par(f"b{t}_mlp2_b", [CH, 1], dt.float32)
    out_ext = nc.declare_dram_parameter("out", [C2, 1024], dt.float32, isOutput=True)

    qdram = [nc.dram_tensor(f"qdram{t}", [J, II], dt.bfloat16) for t in range(NB)]
    odram = [nc.dram_tensor(f"odram{t}", [CH, 1024], dt.bfloat16) for t in range(NB)]
    agin_k = [nc.dram_tensor(f"agin_k{t}", [J, II], dt.bfloat16) for t in range(NB)]
    agout_k = [nc.dram_tensor(f"agout_k{t}", [4 * J, II], dt.bfloat16) for t in range(NB)]
    agin_va = [nc.dram_tensor(f"agin_va{t}", [CH, 1024], dt.bfloat16) for t in range(NB)]
    agout_va = [nc.dram_tensor(f"agout_va{t}", [4 * CH, 1024], dt.bfloat16) for t in range(NB)]
    agin_vb = [nc.dram_tensor(f"agin_vb{t}", [CH, 1024], dt.bfloat16) for t in range(NB)]
    agout_vb = [nc.dram_tensor(f"agout_vb{t}", [4 * CH, 1024], dt.bfloat16) for t in range(NB)]
    dummy_in = nc.dram_tensor("dummy_in", [1, 64], dt.float32)
    dummy_out = nc.dram_tensor("dummy_out", [8, 64], dt.float32, addr_space="Shared")

    RG = [[0, 1, 2, 3], [4, 5, 6, 7]]
    RG8 = [list(range(8))]
    Pool = mybir.EngineType.Pool

    with tile.TileContext(nc) as tc:
        with (
            tc.tile_pool(name="wp", bufs=1) as wp,
            tc.tile_pool(name="sp", bufs=2) as sp,
            tc.tile_pool(name="ps", bufs=2, space="PSUM") as ps_pool,
            tc.tile_pool(name="ps1", bufs=1, space="PSUM") as ps1_pool,
        ):
            # dummy collective to absorb ncfw init (overlaps weight loads)
            zt = wp.tile([1, 64], dt.float32, name="zt")
            nc.vector.memset(zt[:], 0.0)
            nc.gpsimd.dma_start(dummy_in[:], zt[:])
            nc.gpsimd.collective_compute(
                "AllGather", ALU.bypass, replica_groups=RG8,
                ins=[dummy_in[:].opt()], outs=[dummy_out[:].opt()])

            # per-quarter dynamic row offsets: 16*qh + 4*(pid % 4)
            pid = nc.partition_id(engines=[Pool])
            regq = []
            for qh in range(4):
                r = nc.alloc_registers(f"regq{qh}", engines=[Pool])
                nc.regs_alu(r, pid, 4, op=ALU.mod)
                nc.regs_alu(r, r, 4, op=ALU.mult)
                nc.regs_alu(r, r, 16 * qh, op=ALU.add)
                regq.append(nc.snap(r, donate=True, min_val=16 * qh, max_val=16 * qh + 12))

            def wload(name, rows, cols, d=dt.bfloat16):
                ts = []
                for i in range((rows + 127) // 128):
                    t_ = wp.tile([min(128, rows - 128 * i), cols], d, name=f"w_{name}_{i}")
                    nc.sync.dma_start(t_[:], P[name][128 * i:min(rows, 128 * i + 128), :])
                    ts.append(t_)
                return ts

            cv1_wt = wload("cv1_wt", C1, CH)
            cv2_wt = wload("cv2_wt", 3 * CH, C2)
            cv1_b = wload("cv1_b", CH, 1, dt.float32)
            cv2_b = wload("cv2_b", C2, 1, dt.float32)
            gamma = wload("gamma", C2, 1, dt.float32)
            WB = []
            for t in range(NB):
                d_ = {}
                for k, rows, cols, dd in [
                    ("qk_wt", CH, 2 * CH, dt.bfloat16), ("v_wt", CH, CH, dt.bfloat16),
                    ("proj_wt", CH, CH, dt.bfloat16), ("mlp1_wt", CH, 2 * CH, dt.bfloat16),
                    ("mlp2_wt", 2 * CH, CH, dt.bfloat16),
                    ("qb", CH, 1, dt.float32), ("kb", CH, 1, dt.float32),
                    ("v_b", CH, 1, dt.float32), ("vb_d", 8 * 128, 1, dt.float32),
                    ("pe_w", CH, 25, dt.float32), ("pe_b", CH, 1, dt.float32),
                    ("proj_b", CH, 1, dt.float32), ("mlp1_b", 2 * CH, 1, dt.float32),
                    ("mlp2_b", CH, 1, dt.float32),
                ]:
                    d_[k] = wload(f"b{t}_{k}", rows, cols, dd)
                WB.append(d_)

            ones32 = wp.tile([128, 32], dt.bfloat16, name="ones32")
            nc.vector.memset(ones32[:], 1.0)

            ybf = [[wp.tile([128, 1024], dt.bfloat16, name=f"ybf{t}_{m}") for m in range(2)]
                   for t in range(NB + 1)]
            yf32 = [wp.tile([128, 1024], dt.float32, name=f"yf32_{m}") for m in range(2)]
            v4fp = [wp.tile([128, 68 * 64], dt.bfloat16, name=f"v4fp{m}") for m in range(2)]
            for m in range(2):
                nc.vector.memset(v4fp[m][:, 0:128], 0.0)
                nc.vector.memset(v4fp[m][:, 66 * 64:68 * 64], 0.0)

            xbf = []
            for c in range(4):
                xb = sp.tile([128, 1024], dt.bfloat16, tag="hbf", bufs=4, name="xbf")
                nc.sync.dma_start(xb[:], x_bf[128 * c:128 * c + 128, :])
                xbf.append(xb)

            # cv1
            for m in range(2):
                for n_ in range(2):
                    ps = ps_pool.tile([128, 512], dt.float32, tag="ps_s0", bufs=2, name="ps_cv1")
                    for cc in range(4):
                        nc.tensor.matmul(ps[:], cv1_wt[cc][:, 128 * m:128 * m + 128],
                                         xbf[cc][:, 512 * n_:512 * n_ + 512],
                                         start=(cc == 0), stop=(cc == 3))
                    nc.scalar.activation(yf32[m][:, 512 * n_:512 * n_ + 512], ps[:],
                                         AF.Silu, bias=cv1_b[m][:])
                nc.vector.tensor_copy(ybf[0][m][:], yf32[m][:])

            # ================= blocks =================
            for t in range(NBB):
                Wt = WB[t]
                ycur = ybf[t]
                if STAGE == 0:
                    for m in range(2):
                        nc.vector.tensor_copy(ybf[t + 1][m][:], yf32[m][:])
                    continue

                # k convs first -> scatter -> AG_k (so the gather overlaps q/v convs)
                for qh in (range(4) if STAGE >= 1 else []):
                    for pg in range(2):
                        psk = ps_pool.tile([128, 256], dt.float32, tag="ps_s1", bufs=2, name="psk")
                        for cc in range(2):
                            rhs = ycur[cc][:, 256 * qh:256 * qh + 256]
                            for hf in range(2):
                                cg = 2 * pg + hf
                                nc.tensor.matmul(
                                    psk[64 * hf:64 * hf + 64, :],
                                    Wt["qk_wt"][cc][:, 128 * cg + 64:128 * cg + 128], rhs,
                                    start=(cc == 0), stop=(cc == 1), tile_position=(0, 64 * hf))
                        sk = sp.tile([128, 256], dt.bfloat16, tag="stg", bufs=6, name="sk")
                        nc.vector.tensor_scalar(sk[:], psk[:], Wt["kb"][pg][:], None, op0=ALU.add)
                        d4 = agin_k[t].rearrange("(g h u q) j -> g h u q j", g=4, h=8, u=8)
                        nc.gpsimd.dma_start(d4[2 * pg:2 * pg + 2, :, :, qh, :], sk[:])

                if STAGE >= 1:
                    nc.gpsimd.collective_compute(
                        "AllGather", ALU.bypass, replica_groups=RG,
                        ins=[agin_k[t][:].opt()], outs=[agout_k[t][:].opt()])

                # q convs -> local d-layout scatter
                for qh in (range(4) if STAGE >= 1 else []):
                    for pg in range(2):
                        psq = ps_pool.tile([128, 256], dt.float32, tag="ps_s0", bufs=2, name="psq")
                        for cc in range(2):
                            rhs = ycur[cc][:, 256 * qh:256 * qh + 256]
                            for hf in range(2):
                                cg = 2 * pg + hf
                                nc.tensor.matmul(
                                    psq[64 * hf:64 * hf + 64, :],
                                    Wt["qk_wt"][cc][:, 128 * cg:128 * cg + 64], rhs,
                                    start=(cc == 0), stop=(cc == 1), tile_position=(0, 64 * hf))
                        sq = sp.tile([128, 256], dt.bfloat16, tag="stg", bufs=6, name="sq")
                        nc.vector.tensor_scalar(sq[:], psq[:], Wt["qb"][pg][:], None, op0=ALU.add)
                        d4q = qdram[t].rearrange("(g h u q) j -> g h u q j", g=4, h=8, u=8)
                        nc.gpsimd.dma_start(d4q[2 * pg:2 * pg + 2, :, :, qh, :], sq[:])

                # vT conv (j on partitions); staging (jj, c')
                svt = [sp.tile([128, 1024], dt.bfloat16, tag=f"svt{jt}", name="svt")
                       for jt in range(2)]
                for qh in (range(4) if STAGE >= 1 else []):
                    for jt in range(2):
                        psv = ps_pool.tile([128, 256], dt.float32, tag="ps_s0", bufs=2, name="psv")
                        for cc in range(2):
                            nc.tensor.matmul(
                                psv[:],
                                ycur[cc][:, 256 * qh + 128 * jt:256 * qh + 128 * jt + 128],
                                Wt["v_wt"][cc][:], start=(cc == 0), stop=(cc == 1))
                        for gg in range(4):
                            dstv = svt[jt].rearrange("p (g h u q) -> p g h u q",
                                                     g=4, h=8, u=8)[:, gg, :, :, qh]
                            nc.vector.tensor_copy(
                                dstv, psv.rearrange("p (g h u) -> p g h u",
                                                    g=4, h=8)[:, gg, :, :])
                if STAGE >= 1:
                    nc.gpsimd.dma_start(agin_va[t][0:128, :], svt[0][:])
                    nc.gpsimd.dma_start(agin_va[t][128:256, :], svt[1][:])
                    nc.gpsimd.collective_compute(
                        "AllGather", ALU.bypass, replica_groups=RG,
                        ins=[agin_va[t][:].opt()], outs=[agout_va[t][:].opt()])

                # v4 conv in channel layout (own positions)
                for m in (range(2) if STAGE >= 1 else []):
                    for n_ in range(2):
                        psc = ps_pool.tile([128, 512], dt.float32, tag="ps_s1", bufs=2, name="psc")
                        for cc in range(2):
                            nc.tensor.matmul(psc[:], Wt["v_wt"][cc][:, 128 * m:128 * m + 128],
                                             ycur[cc][:, 512 * n_:512 * n_ + 512],
                                             start=(cc == 0), stop=(cc == 1))
                        sv4 = sp.tile([128, 512], dt.bfloat16, tag="sv4", name="sv4")
                        nc.vector.tensor_scalar(sv4[:], psc[:], Wt["v_b"][m][:], None, op0=ALU.add)
                        nc.gpsimd.dma_start(
                            agin_vb[t][128 * m:128 * m + 128,
                                       512 * n_:512 * n_ + 512], sv4[:])

                if STAGE >= 1:
                    nc.gpsimd.collective_compute(
                        "AllGather", ALU.bypass, replica_groups=RG,
                        ins=[agin_vb[t][:].opt()], outs=[agout_vb[t][:].opt()])

                # read back K, Q, vT
                kt_, qt_, vt_ = [], [], []
                agk4 = agout_k[t].rearrange("(r p) j -> r p j", r=4)
                for tt in (range(8) if STAGE >= 2 else []):
                    ktile = sp.tile([128, 1024], dt.bfloat16, tag=f"kt{tt}", bufs=1, name="ktile")
                    nc.scalar.dma_start(
                        ktile[:], agk4[:, 128 * tt:128 * tt + 128, :].transpose([1, 0, 2]))
                    kt_.append(ktile)
                    qtile = sp.tile([128, 256], dt.bfloat16, tag=f"qt{tt}", bufs=1, name="qtile")
                    nc.sync.dma_start(qtile[:], qdram[t][128 * tt:128 * tt + 128, :])
                    qt_.append(qtile)
                    vtile = sp.tile([128, 1024], dt.bfloat16, tag=f"vt{tt}", bufs=1, name="vtile")
                    nc.scalar.dma_start(vtile[:], agout_va[t][128 * tt:128 * tt + 128, :])
                    vt_.append(vtile)

                # v4full (channel layout, all spatial rows) into padded buffer
                agv4 = agout_vb[t].rearrange("(r c) n -> r c n", r=4)
                for m in (range(2) if STAGE >= 2 else []):
                    for qh in range(4):
                        src = agv4[:, 128 * m:128 * m + 128,
                                   256 * qh:256 * qh + 256]
                        src = src.rearrange("r c (w x) -> r c w x", x=64).transpose([1, 0, 2, 3])
                        nc.scalar.dma_start(
                            v4fp[m][:, 64 * (16 * qh + 2):64 * (16 * qh + 2) + 1024], src)

                # dynamic gather of own rows +-2 per quarter
                v4e = [sp.tile([128, 2048], dt.bfloat16, tag=f"v4e{m}", bufs=1, name="v4e")
                       for m in range(2)]
                for m in (range(2) if STAGE >= 3 else []):
                    v3 = v4fp[m].rearrange("p (r x) -> p r x", x=64)
                    for qh in range(4):
                        nc.gpsimd.dma_start(v4e[m][:, 512 * qh:512 * qh + 512],
                                            v3[:, bass.ds(regq[qh], 8), :])

                # depthwise 5x5
                pp = [sp.tile([128, 1024], dt.bfloat16, tag=f"pp{m}", bufs=1, name="pp")
                      for m in range(2)]
                if STAGE < 3:
                    for m in range(2):
                        nc.vector.memset(pp[m][:], 0.0)
                for m in (range(2) if STAGE >= 3 else []):
                    o4 = pp[m].rearrange("p (q r x) -> p q r x", q=4, r=4)
                    i4 = v4e[m].rearrange("p (q r x) -> p q r x", q=4, r=8)
                    ctap = 12
                    for qh in range(4):
                        nc.vector.tensor_scalar(o4[:, qh, :, :], i4[:, qh, 2:6, :],
                                                Wt["pe_w"][m][:, ctap:ctap + 1],
                                                Wt["pe_b"][m][:], op0=ALU.mult, op1=ALU.add)
                    for dy in range(5):
                        for dx in range(5):
                            if dy == 2 and dx == 2:
                                continue
                            tap = dy * 5 + dx
                            a = max(0, 2 - dx); b = min(64, 66 - dx)
                            for qh in range(4):
                                nc.vector.scalar_tensor_tensor(
                                    o4[:, qh, :, a:b],
                                    i4[:, qh, dy:dy + 4, a - 2 + dx:b - 2 + dx],
                                    Wt["pe_w"][m][:, tap:tap + 1],
                                    o4[:, qh, :, a:b], op0=ALU.mult, op1=ALU.add)

                # attention per (g, hh)
                for g in (range(4) if STAGE >= 4 else []):
                    for hh in range(2):
                        tt = 2 * g + hh
                        ps_o = ps_l = None
                        if ATT >= 2:
                            ps_o = ps1_pool.tile([128, 256], dt.float32, tag="ps_o", name="ps_o")
                            ps_l = ps1_pool.tile([128, 256], dt.float32, tag="ps_l", name="ps_l")
                        for jp in range(4):
                            us2 = []
                            for h2 in range(4):
                                ps_s1 = ps_pool.tile([128, 512], dt.float32,
                                                     tag=f"ps_s{h2}", bufs=(2 if h2 < 2 else 1), name="ps_s1")
                                for sub in range(2):
                                    jt = 2 * jp + sub
                                    nc.tensor.matmul(
                                        ps_s1[:, 256 * sub:256 * sub + 256],
                                        kt_[tt][32 * h2:32 * h2 + 32, 128 * jt:128 * jt + 128],
                                        qt_[tt][32 * h2:32 * h2 + 32, :],
                                        start=True, stop=True, tile_position=(32 * h2, 0))
                                u2 = sp.tile([128, 512], dt.bfloat16, tag=f"u{jp}h{h2}",
                                             bufs=1, name="u2")
                                nc.scalar.activation(u2[:], ps_s1[:], AF.Exp, scale=SCALE)
                                us2.append(u2)
                            for h2 in range(4):
                                c0 = 256 * g + 32 * (4 * hh + h2)
                                for sub in range(2):
                                    jt = 2 * jp + sub
                                    nc.tensor.matmul(
                                        ps_o[32 * h2:32 * h2 + 32, :],
                                        vt_[jt][:, c0:c0 + 32],
                                        us2[h2][:, 256 * sub:256 * sub + 256],
                                        start=(jp == 0 and sub == 0),
                                        stop=(jp == 3 and sub == 1), tile_position=(0, 32 * h2))
                                    nc.tensor.matmul(
                                        ps_l[32 * h2:32 * h2 + 32, :],
                                        ones32[:],
                                        us2[h2][:, 256 * sub:256 * sub + 256],
                                        start=(jp == 0 and sub == 0),
                                        stop=(jp == 3 and sub == 1), tile_position=(0, 32 * h2))
                        if ATT == 1:
                            continue
                        if ATT == 2:
                            o_dbg = sp.tile([128, 256], dt.float32, tag="o_f", name="o_dbg")
                            nc.vector.tensor_copy(o_dbg[:], ps_o[:])
                            nc.gpsimd.dma_start(odram[t][32 * tt:32 * tt + 1, 0:4], o_dbg[0:1, 0:4])
                            continue
                        r_sb = sp.tile([128, 256], dt.float32, tag="r_sb", name="r_sb")
                        nc.vector.reciprocal(r_sb[:], ps_l[:])
                        o_f = sp.tile([128, 256], dt.float32, tag="o_f", name="o_f")
                        nc.vector.tensor_tensor(o_f[:], ps_o[:], r_sb[:], op=ALU.mult)
                        o_st = sp.tile([128, 256], dt.bfloat16, tag="o_st", name="o_st")
                        nc.vector.tensor_scalar(o_st[:], o_f[:], Wt["vb_d"][tt][:], None,
                                                op0=ALU.add)
                        if ATT == 3:
                            nc.gpsimd.dma_start(odram[t][32 * tt:32 * tt + 1, 0:4], o_st[0:1, 0:4])
                        else:
                            for h2 in range(4):
                                dsto = odram[t][32 * tt + 8 * h2:32 * tt + 8 * h2 + 8, :] \
                                    .rearrange("u (q j) -> u q j", q=4)
                                nc.gpsimd.dma_start(dsto, o_st[32 * h2:32 * h2 + 32, :])

                # o + pp -> proj -> residual
                oc, ao = [], []
                for m in range(2):
                    oct = sp.tile([128, 1024], dt.bfloat16, tag=f"oc{m}", bufs=1, name="oct")
                    if STAGE >= 4:
                        nc.sync.dma_start(oct[:], odram[t][128 * m:128 * m + 128, :])
                    else:
                        nc.vector.memset(oct[:], 0.0)
                    oc.append(oct)
                for m in range(2):
                    aot = sp.tile([128, 1024], dt.bfloat16, tag=f"ao{m}", bufs=1, name="aot")
                    nc.vector.tensor_tensor(aot[:], oc[m][:], pp[m][:], op=ALU.add)
                    ao.append(aot)
                ymid = []
                for m in range(2):
                    ym = sp.tile([128, 1024], dt.bfloat16, tag=f"ym{m}", bufs=1, name="ym")
                    for n_ in range(2):
                        psp = ps_pool.tile([128, 512], dt.float32, tag="ps_s0", bufs=2, name="psp")
                        for cc in range(2):
                            nc.tensor.matmul(psp[:], Wt["proj_wt"][cc][:, 128 * m:128 * m + 128],
                                             ao[cc][:, 512 * n_:512 * n_ + 512],
                                             start=(cc == 0), stop=(cc == 1))
                        nc.vector.scalar_tensor_tensor(
                            yf32[m][:, 512 * n_:512 * n_ + 512], psp[:], Wt["proj_b"][m][:],
                            yf32[m][:, 512 * n_:512 * n_ + 512], op0=ALU.add, op1=ALU.add)
                    nc.vector.tensor_copy(ym[:], yf32[m][:])
                    ymid.append(ym)

                # mlp
                hbf = []
                for mt in range(4):
                    hb = sp.tile([128, 1024], dt.bfloat16, tag="hbf", bufs=4, name="hbf")
                    for n_ in range(2):
                        ps1_ = ps_pool.tile([128, 512], dt.float32, tag="ps_s1", bufs=2, name="ps_mlp1")
                        for cc in range(2):
                            nc.tensor.matmul(ps1_[:], Wt["mlp1_wt"][cc][:, 128 * mt:128 * mt + 128],
                                             ymid[cc][:, 512 * n_:512 * n_ + 512],
                                             start=(cc == 0), stop=(cc == 1))
                        nc.scalar.activation(hb[:, 512 * n_:512 * n_ + 512], ps1_[:],
                                             AF.Silu, bias=Wt["mlp1_b"][mt][:])
                    hbf.append(hb)
                for m in range(2):
                    for n_ in range(2):
                        ps2_ = ps_pool.tile([128, 512], dt.float32, tag="ps_s0", bufs=2, name="ps_mlp2")
                        for cc in range(4):
                            nc.tensor.matmul(ps2_[:], Wt["mlp2_wt"][cc][:, 128 * m:128 * m + 128],
                                             hbf[cc][:, 512 * n_:512 * n_ + 512],
                                             start=(cc == 0), stop=(cc == 3))
                        nc.vector.scalar_tensor_tensor(
                            yf32[m][:, 512 * n_:512 * n_ + 512], ps2_[:], Wt["mlp2_b"][m][:],
                            yf32[m][:, 512 * n_:512 * n_ + 512], op0=ALU.add, op1=ALU.add)
                    nc.vector.tensor_copy(ybf[t + 1][m][:], yf32[m][:])

            # cv2 + delta out (final x-residual added host-side)
            ycat = [ybf[0][0], ybf[0][1], ybf[min(2, NBB)][0], ybf[min(2, NBB)][1], ybf[NBB][0], ybf[NBB][1]]
            for m in range(4):
                for n_ in range(2):
                    psf = ps_pool.tile([128, 512], dt.float32, tag="ps_s1", bufs=2, name="ps_cv2")
                    for cc in range(6):
                        nc.tensor.matmul(psf[:], cv2_wt[cc][:, 128 * m:128 * m + 128],
                                         ycat[cc][:, 512 * n_:512 * n_ + 512],
                                         start=(cc == 0), stop=(cc == 5))
                    os_ = sp.tile([128, 512], dt.float32, tag="os", name="os_")
                    nc.scalar.activation(os_[:], psf[:], AF.Silu, bias=cv2_b[m][:])
                    dl = sp.tile([128, 512], dt.float32, tag="dl", name="dl")
                    nc.vector.tensor_scalar(dl[:], os_[:], gamma[m][:], None, op0=ALU.mult)
                    nc.sync.dma_start(out_ext[128 * m:128 * m + 128, 512 * n_:512 * n_ + 512],
                                      dl[:])

    nc.compile()
    return nc


def _prep_inputs(x, params):
    xf = np.asarray(x, np.float32).reshape(B, C1, N)
    f32 = lambda a: np.ascontiguousarray(np.asarray(a, np.float32))
    bfc = lambda a: np.ascontiguousarray(np.asarray(a, np.float32).astype(bf16))
    col = lambda a: f32(a).reshape(-1, 1)

    shared = {
        "cv1_wt": bfc(np.asarray(params["cv1_w"]).T), "cv1_b": col(params["cv1_b"]),
        "cv2_wt": bfc(np.asarray(params["cv2_w"]).T), "cv2_b": col(params["cv2_b"]),
        "gamma": col(params["gamma"]),
    }
    blocks = [bp for stack in params["blocks"] for bp in stack]
    for t, bp in enumerate(blocks):
        qkw = np.asarray(bp["qk_w"], np.float32)
        shared[f"b{t}_qk_wt"] = bfc(qkw.T)
        qkb = np.asarray(bp["qk_b"], np.float32)
        qb = np.zeros((CH,), np.float32); kb = np.zeros((CH,), np.float32)
        for pg in range(2):
            for half in range(2):
                cg = 2 * pg + half
                qb[128 * pg + 64 * half:128 * pg + 64 * half + 64] = \
                    qkb[128 * cg:128 * cg + 64]
                kb[128 * pg + 64 * half:128 * pg + 64 * half + 64] = \
                    qkb[128 * cg + 64:128 * cg + 128]
        shared[f"b{t}_qb"] = qb.reshape(-1, 1); shared[f"b{t}_kb"] = kb.reshape(-1, 1)
        shared[f"b{t}_v_wt"] = bfc(np.asarray(bp["v_w"]).T)
        vb = np.asarray(bp["v_b"], np.float32)
        shared[f"b{t}_v_b"] = vb.reshape(-1, 1)
        vbd = np.zeros((8, 128), np.float32)
        for g in range(4):
            for hh in range(2):
                for h2 in range(4):
                    for uu in range(8):
                        for qh in range(4):
                            vbd[2 * g + hh, 32 * h2 + 4 * uu + qh] = \
                                vb[64 * g + 8 * (4 * hh + h2) + uu]
        shared[f"b{t}_vb_d"] = vbd.reshape(-1, 1)
        shared[f"b{t}_pe_w"] = f32(np.asarray(bp["pe_w"]).reshape(CH, 25))
        shared[f"b{t}_pe_b"] = col(bp["pe_b"])
        shared[f"b{t}_proj_wt"] = bfc(np.asarray(bp["proj_w"]).T)
        shared[f"b{t}_proj_b"] = col(bp["proj_b"])
        shared[f"b{t}_mlp1_wt"] = bfc(np.asarray(bp["mlp1_w"]).T)
        shared[f"b{t}_mlp1_b"] = col(bp["mlp1_b"])
        shared[f"b{t}_mlp2_wt"] = bfc(np.asarray(bp["mlp2_w"]).T)
        shared[f"b{t}_mlp2_b"] = col(bp["mlp2_b"])

    in_maps = []
    for core in range(NC):
        b, s = core // 4, core % 4
        cols = (np.arange(4)[:, None] * 1024 + 256 * s + np.arange(256)[None, :]).reshape(-1)
        m = dict(shared)
        m["x_bf"] = np.ascontiguousarray(xf[b][:, cols].astype(bf16))
        in_maps.append(m)
    return in_maps, xf


def _run(x, params, trace=False):
    if "nc" not in _cache:
        _cache["nc"] = _build()
    nc = _cache["nc"]
    in_maps, xf = _prep_inputs(x, params)
    res = run_bass_kernel_spmd(nc, in_maps, core_ids=list(range(NC)), trace=trace)
    out = np.empty((B, C2, N), np.float32)
    for core in range(NC):
        b, s = core // 4, core % 4
        cols = (np.arange(4)[:, None] * 1024 + 256 * s + np.arange(256)[None, :]).reshape(-1)
        out[b][:, cols] = xf[b][:, cols] + res.results[core]["out"]
    return out.reshape(B, C2, H, W), res


def kernel(x, params):
    out, _ = _run(x, params, trace=False)
    return out


# revision 25
# speedup vs baseline: 1.2513x; 1.0175x over previous
"""A2C2f Trainium2 kernel: 8-core SPMD via bass/tile.

Sharding: core = b*4 + s.  b = batch (2), s = sequence-slice (4).
The reference's area reshape on (B, C, N) row-major arrays maps
  (b, c, n) -> group g = b*4 + c//128 (qk) / c//64 (v), head h, d = 4*uu + qhat,
  sequence pos j = n % 1024   (qhat = n//1024 spatial quarter, uu low channel bits).
Each core owns sequence slice j in [256*s, 256*s+256) for ALL groups/heads of its
batch => queries local, keys/values AllGathered (2 collectives per block).
Attention: S^T = k^T q row-tiled 4 heads/matmul (K=32); exp on ACT (scale folded);
L = ones-reduce col-tiled; out2 col-tiled (M=32); normalize = DVE reciprocal+mul.
Depthwise 5x5 PE conv: v4 gathered in channel layout, own rows +-2 selected with a
partition_id-derived register (dynamic DMA), 25 in-place DVE FMAs.
"""
import os, sys, types
sys.path.insert(0, '/opt/trn_rl_repo')
import numpy as np
import ml_dtypes

if "antenv.axon_hooks" not in sys.modules:
    _m = types.ModuleType("antenv.axon_hooks")
    _h = [None]
    _m.set_axon_ntff_profile_hook = lambda h: _h.__setitem__(0, h)
    _m.get_axon_ntff_profile_hook = lambda: _h[0]
    sys.modules["antenv.axon_hooks"] = _m
    try:
        import antenv
        antenv.axon_hooks = _m
        from trn_agent_boot.trn_boot import _ntff_profile_via_ctypes
        _m.set_axon_ntff_profile_hook(_ntff_profile_via_ctypes('/opt/axon/libaxon_pjrt.so'))
    except Exception:
        pass

import concourse.bass as bass
import concourse.bacc as bacc
import concourse.mybir as mybir
import concourse.tile as tile
import concourse.bass_utils as bass_utils
from concourse.bass_utils import run_bass_kernel_spmd

bass_utils.upload_artifacts = lambda tmpdir: "local://skipped"

dt = mybir.dt
AF = mybir.ActivationFunctionType
ALU = mybir.AluOpType
bf16 = ml_dtypes.bfloat16

NC = 8
B, C1, C2, H, W = 2, 512, 512, 64, 64
CH = 256
NB = 4
N = H * W
J = 1024
II = 256
SCALE = 32 ** -0.5
STAGE = int(os.environ.get("A2_STAGE", "9"))
ATT = int(os.environ.get("A2_ATT", "9"))
PSV = int(os.environ.get("A2_PSV", "1"))
NBB = int(os.environ.get("A2_NB", "4"))

_cache = {}


def _build():
    nc = bacc.Bacc("TRN2", target_bir_lowering=False, debug=False, num_devices=NC)

    x_bf = nc.declare_dram_parameter("x_bf", [C1, 1024], dt.bfloat16, isOutput=False)
    P = {}
    def par(name, shape, d=dt.bfloat16):
        P[name] = nc.declare_dram_parameter(name, shape, d, isOutput=False)
    par("cv1_wt", [C1, CH]); par("cv1_b", [CH, 1], dt.float32)
    par("cv2_wt", [3 * CH, C2]); par("cv2_b", [C2, 1], dt.float32)
    par("gamma", [C2, 1], dt.float32)
    for t in range(NB):
        par(f"b{t}_qk_wt", [CH, 2 * CH])
        par(f"b{t}_qb", [CH, 1], dt.float32); par(f"b{t}_kb", [CH, 1], dt.float32)
        par(f"b{t}_v_wt", [CH, CH]); par(f"b{t}_v_b", [CH, 1], dt.float32)
        par(f"b{t}_vb_d", [8 * 128, 1], dt.float32)
        par(f"b{t}_pe_w", [CH, 25], dt.float32); par(f"b{t}_pe_b", [CH, 1], dt.float32)
        par(f"b{t}_proj_wt", [CH, CH]); par(f"b{t}_proj_b", [CH, 1], dt.float32)
        par(f"b{t}_mlp1_wt", [CH, 2 * CH]); par(f"b{t}_mlp1_b", [2 * CH, 1], dt.float32)
        par(f"b{t}_mlp2_wt", [2 * CH, CH]); par(f"b{t}_mlp2_b", [CH, 1], dt.float32)
    out_ext = nc.declare_dram_parameter("out", [C2, 1024], dt.float32, isOutput=True)

    qdram = [nc.dram_tensor(f"qdram{t}", [J, II], dt.bfloat16) for t in range(NB)]
    odram = [nc.dram_tensor(f"odram{t}", [CH, 1024], dt.bfloat16) for t in range(NB)]
    agin_k = [nc.dram_tensor(f"agin_k{t}", [J, II], dt.bfloat16) for t in range(NB)]
    agout_k = [nc.dram_tensor(f"agout_k{t}", [4 * J, II], dt.bfloat16) for t in range(NB)]
    agin_va = [nc.dram_tensor(f"agin_va{t}", [CH, 1024], dt.bfloat16) for t in range(NB)]
    agout_va = [nc.dram_tensor(f"agout_va{t}", [4 * CH, 1024], dt.bfloat16) for t in range(NB)]
    agin_vb = [nc.dram_tensor(f"agin_vb{t}", [CH, 1024], dt.bfloat16) for t in range(NB)]
    agout_vb = [nc.dram_tensor(f"agout_vb{t}", [4 * CH, 1024], dt.bfloat16) for t in range(NB)]
    dummy_in = nc.dram_tensor("dummy_in", [1, 64], dt.float32)
    dummy_out = nc.dram_tensor("dummy_out", [8, 64], dt.float32, addr_space="Shared")

    RG = [[0, 1, 2, 3], [4, 5, 6, 7]]
    RG8 = [list(range(8))]
    Pool = mybir.EngineType.Pool

    with tile.TileContext(nc) as tc:
        with (
            tc.tile_pool(name="wp", bufs=1) as wp,
            tc.tile_pool(name="sp", bufs=2) as sp,
            tc.tile_pool(name="ps", bufs=2, space="PSUM") as ps_pool,
            tc.tile_pool(name="ps1", bufs=1, space="PSUM") as ps1_pool,
        ):
            # dummy collective to absorb ncfw init (overlaps weight loads)
            zt = wp.tile([1, 64], dt.float32, name="zt")
            nc.vector.memset(zt[:], 0.0)
            nc.gpsimd.dma_start(dummy_in[:], zt[:])
            nc.gpsimd.collective_compute(
                "AllGather", ALU.bypass, replica_groups=RG8,
                ins=[dummy_in[:].opt()], outs=[dummy_out[:].opt()])

            # per-quarter dynamic row offsets: 16*qh + 4*(pid % 4)
            pid = nc.partition_id(engines=[Pool])
            regq = []
            for qh in range(4):
                r = nc.alloc_registers(f"regq{qh}", engines=[Pool])
                nc.regs_alu(r, pid, 4, op=ALU.mod)
                nc.regs_alu(r, r, 4, op=ALU.mult)
                nc.regs_alu(r, r, 16 * qh, op=ALU.add)
                regq.append(nc.snap(r, donate=True, min_val=16 * qh, max_val=16 * qh + 12))

            def wload(name, rows, cols, d=dt.bfloat16):
                ts = []
                for i in range((rows + 127) // 128):
                    t_ = wp.tile([min(128, rows - 128 * i), cols], d, name=f"w_{name}_{i}")
                    nc.sync.dma_start(t_[:], P[name][128 * i:min(rows, 128 * i + 128), :])
                    ts.append(t_)
                return ts

            cv1_wt = wload("cv1_wt", C1, CH)
            cv2_wt = wload("cv2_wt", 3 * CH, C2)
            cv1_b = wload("cv1_b", CH, 1, dt.float32)
            cv2_b = wload("cv2_b", C2, 1, dt.float32)
            gamma = wload("gamma", C2, 1, dt.float32)
            WB = []
            for t in range(NB):
                d_ = {}
                for k, rows, cols, dd in [
                    ("qk_wt", CH, 2 * CH, dt.bfloat16), ("v_wt", CH, CH, dt.bfloat16),
                    ("proj_wt", CH, CH, dt.bfloat16), ("mlp1_wt", CH, 2 * CH, dt.bfloat16),
                    ("mlp2_wt", 2 * CH, CH, dt.bfloat16),
                    ("qb", CH, 1, dt.float32), ("kb", CH, 1, dt.float32),
                    ("v_b", CH, 1, dt.float32), ("vb_d", 8 * 128, 1, dt.float32),
                    ("pe_w", CH, 25, dt.float32), ("pe_b", CH, 1, dt.float32),
                    ("proj_b", CH, 1, dt.float32), ("mlp1_b", 2 * CH, 1, dt.float32),
                    ("mlp2_b", CH, 1, dt.float32),
                ]:
                    d_[k] = wload(f"b{t}_{k}", rows, cols, dd)
                WB.append(d_)

            ones32 = wp.tile([128, 32], dt.bfloat16, name="ones32")
            nc.vector.memset(ones32[:], 1.0)

            ybf = [[wp.tile([128, 1024], dt.bfloat16, name=f"ybf{t}_{m}") for m in range(2)]
                   for t in range(NB + 1)]
            yf32 = [wp.tile([128, 1024], dt.float32, name=f"yf32_{m}") for m in range(2)]
            v4fp = [wp.tile([128, 68 * 64], dt.bfloat16, name=f"v4fp{m}") for m in range(2)]
            for m in range(2):
                nc.vector.memset(v4fp[m][:, 0:128], 0.0)
                nc.vector.memset(v4fp[m][:, 66 * 64:68 * 64], 0.0)

            xbf = []
            for c in range(4):
                xb = sp.tile([128, 1024], dt.bfloat16, tag="hbf", bufs=4, name="xbf")
                nc.sync.dma_start(xb[:], x_bf[128 * c:128 * c + 128, :])
                xbf.append(xb)

            # cv1
            for m in range(2):
                for n_ in range(2):
                    ps = ps_pool.tile([128, 512], dt.float32, tag="ps_s0", bufs=2, name="ps_cv1")
                    for cc in range(4):
                        nc.tensor.matmul(ps[:], cv1_wt[cc][:, 128 * m:128 * m + 128],
                                         xbf[cc][:, 512 * n_:512 * n_ + 512],
                                         start=(cc == 0), stop=(cc == 3))
                    nc.scalar.activation(yf32[m][:, 512 * n_:512 * n_ + 512], ps[:],
                                         AF.Silu, bias=cv1_b[m][:])
                nc.vector.tensor_copy(ybf[0][m][:], yf32[m][:])

            # ================= blocks =================
            for t in range(NBB):
                Wt = WB[t]
                ycur = ybf[t]
                if STAGE == 0:
                    for m in range(2):
                        nc.vector.tensor_copy(ybf[t + 1][m][:], yf32[m][:])
                    continue

                # k convs first -> scatter -> AG_k (so the gather overlaps q/v convs)
                for qh in (range(4) if STAGE >= 1 else []):
                    for pg in range(2):
                        psk = ps_pool.tile([128, 256], dt.float32, tag="ps_s1", bufs=2, name="psk")
                        for cc in range(2):
                            rhs = ycur[cc][:, 256 * qh:256 * qh + 256]
                            for hf in range(2):
                                cg = 2 * pg + hf
                                nc.tensor.matmul(
                                    psk[64 * hf:64 * hf + 64, :],
                                    Wt["qk_wt"][cc][:, 128 * cg + 64:128 * cg + 128], rhs,
                                    start=(cc == 0), stop=(cc == 1), tile_position=(0, 64 * hf))
                        sk = sp.tile([128, 256], dt.bfloat16, tag="stg", bufs=6, name="sk")
                        nc.vector.tensor_scalar(sk[:], psk[:], Wt["kb"][pg][:], None, op0=ALU.add)
                        d4 = agin_k[t].rearrange("(g h u q) j -> g h u q j", g=4, h=8, u=8)
                        nc.gpsimd.dma_start(d4[2 * pg:2 * pg + 2, :, :, qh, :], sk[:])

                if STAGE >= 1:
                    nc.gpsimd.collective_compute(
                        "AllGather", ALU.bypass, replica_groups=RG,
                        ins=[agin_k[t][:].opt()], outs=[agout_k[t][:].opt()])

                # q convs -> local d-layout scatter
                for qh in (range(4) if STAGE >= 1 else []):
                    for pg in range(2):
                        psq = ps_pool.tile([128, 256], dt.float32, tag="ps_s0", bufs=2, name="psq")
                        for cc in range(2):
                            rhs = ycur[cc][:, 256 * qh:256 * qh + 256]
                            for hf in range(2):
                                cg = 2 * pg + hf
                                nc.tensor.matmul(
                                    psq[64 * hf:64 * hf + 64, :],
                                    Wt["qk_wt"][cc][:, 128 * cg:128 * cg + 64], rhs,
                                    start=(cc == 0), stop=(cc == 1), tile_position=(0, 64 * hf))
                        sq = sp.tile([128, 256], dt.bfloat16, tag="stg", bufs=6, name="sq")
                        nc.vector.tensor_scalar(sq[:], psq[:], Wt["qb"][pg][:], None, op0=ALU.add)
                        d4q = qdram[t].rearrange("(g h u q) j -> g h u q j", g=4, h=8, u=8)
                        nc.gpsimd.dma_start(d4q[2 * pg:2 * pg + 2, :, :, qh, :], sq[:])

                # vT conv (j on partitions); staging (jj, c')
                svt = [sp.tile([128, 1024], dt.bfloat16, tag=f"svt{jt}", name="svt")
                       for jt in range(2)]
                for qh in (range(4) if STAGE >= 1 else []):
                    for jt in range(2):
                        psv = ps_pool.tile([128, 256], dt.float32, tag="ps_s0", bufs=2, name="psv")
                        for cc in range(2):
                            nc.tensor.matmul(
                                psv[:],
                                ycur[cc][:, 256 * qh + 128 * jt:256 * qh + 128 * jt + 128],
                                Wt["v_wt"][cc][:], start=(cc == 0), stop=(cc == 1))
                        for gg in range(4):
                            dstv = svt[jt].rearrange("p (g h u q) -> p g h u q",
                                                     g=4, h=8, u=8)[:, gg, :, :, qh]
                            nc.vector.tensor_copy(
                                dstv, psv.rearrange("p (g h u) -> p g h u",
                                                    g=4, h=8)[:, gg, :, :])
                if STAGE >= 1:
                    nc.gpsimd.dma_start(agin_va[t][0:128, :], svt[0][:])
                    nc.gpsimd.dma_start(agin_va[t][128:256, :], svt[1][:])
                    nc.gpsimd.collective_compute(
                        "AllGather", ALU.bypass, replica_groups=RG,
                        ins=[agin_va[t][:].opt()], outs=[agout_va[t][:].opt()])

                # v4 conv in channel layout (own positions)
                for m in (range(2) if STAGE >= 1 else []):
                    for n_ in range(2):
                        psc = ps_pool.tile([128, 512], dt.float32, tag="ps_s1", bufs=2, name="psc")
                        for cc in range(2):
                            nc.tensor.matmul(psc[:], Wt["v_wt"][cc][:, 128 * m:128 * m + 128],
                                             ycur[cc][:, 512 * n_:512 * n_ + 512],
                                             start=(cc == 0), stop=(cc == 1))
                        sv4 = sp.tile([128, 512], dt.bfloat16, tag="sv4", name="sv4")
                        nc.vector.tensor_scalar(sv4[:], psc[:], Wt["v_b"][m][:], None, op0=ALU.add)
                        nc.gpsimd.dma_start(
                            agin_vb[t][128 * m:128 * m + 128,
                                       512 * n_:512 * n_ + 512], sv4[:])

                if STAGE >= 1:
                    nc.gpsimd.collective_compute(
                        "AllGather", ALU.bypass, replica_groups=RG,
                        ins=[agin_vb[t][:].opt()], outs=[agout_vb[t][:].opt()])

                # read back K, Q, vT
                kt_, qt_, vt_ = [], [], []
                agk4 = agout_k[t].rearrange("(r p) j -> r p j", r=4)
                for tt in (range(8) if STAGE >= 2 else []):
                    ktile = sp.tile([128, 1024], dt.bfloat16, tag=f"kt{tt}", bufs=1, name="ktile")
                    nc.sync.dma_start(
                        ktile[:], agk4[:, 128 * tt:128 * tt + 128, :].transpose([1, 0, 2]))
                    kt_.append(ktile)
                    qtile = sp.tile([128, 256], dt.bfloat16, tag=f"qt{tt}", bufs=1, name="qtile")
                    nc.sync.dma_start(qtile[:], qdram[t][128 * tt:128 * tt + 128, :])
                    qt_.append(qtile)
                    vtile = sp.tile([128, 1024], dt.bfloat16, tag=f"vt{tt}", bufs=1, name="vtile")
                    nc.sync.dma_start(vtile[:], agout_va[t][128 * tt:128 * tt + 128, :])
                    vt_.append(vtile)

                # v4full (channel layout, all spatial rows) into padded buffer
                agv4 = agout_vb[t].rearrange("(r c) n -> r c n", r=4)
                for m in (range(2) if STAGE >= 2 else []):
                    for qh in range(4):
                        src = agv4[:, 128 * m:128 * m + 128,
                                   256 * qh:256 * qh + 256]
                        src = src.rearrange("r c (w x) -> r c w x", x=64).transpose([1, 0, 2, 3])
                        nc.sync.dma_start(
                            v4fp[m][:, 64 * (16 * qh + 2):64 * (16 * qh + 2) + 1024], src)

                # dynamic gather of own rows +-2 per quarter
                v4e = [sp.tile([128, 2048], dt.bfloat16, tag=f"v4e{m}", bufs=1, name="v4e")
                       for m in range(2)]
                for m in (range(2) if STAGE >= 3 else []):
                    v3 = v4fp[m].rearrange("p (r x) -> p r x", x=64)
                    for qh in range(4):
                        nc.gpsimd.dma_start(v4e[m][:, 512 * qh:512 * qh + 512],
                                            v3[:, bass.ds(regq[qh], 8), :])

                # depthwise 5x5
                pp = [sp.tile([128, 1024], dt.bfloat16, tag=f"pp{m}", bufs=1, name="pp")
                      for m in range(2)]
                if STAGE < 3:
                    for m in range(2):
                        nc.vector.memset(pp[m][:], 0.0)
                for m in (range(2) if STAGE >= 3 else []):
                    o4 = pp[m].rearrange("p (q r x) -> p q r x", q=4, r=4)
                    i4 = v4e[m].rearrange("p (q r x) -> p q r x", q=4, r=8)
                    ctap = 12
                    for qh in range(4):
                        nc.vector.tensor_scalar(o4[:, qh, :, :], i4[:, qh, 2:6, :],
                                                Wt["pe_w"][m][:, ctap:ctap + 1],
                                                Wt["pe_b"][m][:], op0=ALU.mult, op1=ALU.add)
                    for dy in range(5):
                        for dx in range(5):
                            if dy == 2 and dx == 2:
                                continue
                            tap = dy * 5 + dx
                            a = max(0, 2 - dx); b = min(64, 66 - dx)
                            for qh in range(4):
                                nc.vector.scalar_tensor_tensor(
                                    o4[:, qh, :, a:b],
                                    i4[:, qh, dy:dy + 4, a - 2 + dx:b - 2 + dx],
                                    Wt["pe_w"][m][:, tap:tap + 1],
                                    o4[:, qh, :, a:b], op0=ALU.mult, op1=ALU.add)

                # attention per (g, hh)
                for g in (range(4) if STAGE >= 4 else []):
                    for hh in range(2):
                        tt = 2 * g + hh
                        ps_o = ps_l = None
                        if ATT >= 2:
                            ps_o = ps1_pool.tile([128, 256], dt.float32, tag="ps_o", name="ps_o")
                            ps_l = ps1_pool.tile([128, 256], dt.float32, tag="ps_l", name="ps_l")
                        for jp in range(4):
                            us2 = []
                            for h2 in range(4):
                                ps_s1 = ps_pool.tile([128, 512], dt.float32,
                                                     tag=f"ps_s{h2}", bufs=(2 if h2 < 2 else 1), name="ps_s1")
                                for sub in range(2):
                                    jt = 2 * jp + sub
                                    nc.tensor.matmul(
                                        ps_s1[:, 256 * sub:256 * sub + 256],
                                        kt_[tt][32 * h2:32 * h2 + 32, 128 * jt:128 * jt + 128],
                                        qt_[tt][32 * h2:32 * h2 + 32, :],
                                        start=True, stop=True, tile_position=(32 * h2, 0))
                                u2 = sp.tile([128, 512], dt.bfloat16, tag=f"u{jp}h{h2}",
                                             bufs=1, name="u2")
                                nc.scalar.activation(u2[:], ps_s1[:], AF.Exp, scale=SCALE)
                                us2.append(u2)
                            for h2 in range(4):
                                c0 = 256 * g + 32 * (4 * hh + h2)
                                for sub in range(2):
                                    jt = 2 * jp + sub
                                    nc.tensor.matmul(
                                        ps_o[32 * h2:32 * h2 + 32, :],
                                        vt_[jt][:, c0:c0 + 32],
                                        us2[h2][:, 256 * sub:256 * sub + 256],
                                        start=(jp == 0 and sub == 0),
                                        stop=(jp == 3 and sub == 1), tile_position=(0, 32 * h2))
                                    nc.tensor.matmul(
                                        ps_l[32 * h2:32 * h2 + 32, :],
                                        ones32[:],
                                        us2[h2][:, 256 * sub:256 * sub + 256],
                                        start=(jp == 0 and sub == 0),
                                        stop=(jp == 3 and sub == 1), tile_position=(0, 32 * h2))
                        if ATT == 1:
                            continue
                        if ATT == 2:
                            o_dbg = sp.tile([128, 256], dt.float32, tag="o_f", name="o_dbg")
                            nc.vector.tensor_copy(o_dbg[:], ps_o[:])
                            nc.gpsimd.dma_start(odram[t][32 * tt:32 * tt + 1, 0:4], o_dbg[0:1, 0:4])
                            continue
                        r_sb = sp.tile([128, 256], dt.float32, tag="r_sb", name="r_sb")
                        nc.vector.reciprocal(r_sb[:], ps_l[:])
                        o_f = sp.tile([128, 256], dt.float32, tag="o_f", name="o_f")
                        nc.vector.tensor_tensor(o_f[:], ps_o[:], r_sb[:], op=ALU.mult)
                        o_st = sp.tile([128, 256], dt.bfloat16, tag="o_st", name="o_st")
                        nc.vector.tensor_scalar(o_st[:], o_f[:], Wt["vb_d"][tt][:], None,
                                                op0=ALU.add)
                        if ATT == 3:
                            nc.gpsimd.dma_start(odram[t][32 * tt:32 * tt + 1, 0:4], o_st[0:1, 0:4])
                        else:
                            for h2 in range(4):
                                dsto = odram[t][32 * tt + 8 * h2:32 * tt + 8 * h2 + 8, :] \
                                    .rearrange("u (q j) -> u q j", q=4)
                                nc.gpsimd.dma_start(dsto, o_st[32 * h2:32 * h2 + 32, :])

                # o + pp -> proj -> residual
                oc, ao = [], []
                for m in range(2):
                    oct = sp.tile([128, 1024], dt.bfloat16, tag=f"oc{m}", bufs=1, name="oct")
                    if STAGE >= 4:
                        nc.sync.dma_start(oct[:], odram[t][128 * m:128 * m + 128, :])
                    else:
                        nc.vector.memset(oct[:], 0.0)
                    oc.append(oct)
                for m in range(2):
                    aot = sp.tile([128, 1024], dt.bfloat16, tag=f"ao{m}", bufs=1, name="aot")
                    nc.vector.tensor_tensor(aot[:], oc[m][:], pp[m][:], op=ALU.add)
                    ao.append(aot)
                ymid = []
                for m in range(2):
                    ym = sp.tile([128, 1024], dt.bfloat16, tag=f"ym{m}", bufs=1, name="ym")
                    for n_ in range(2):
                        psp = ps_pool.tile([128, 512], dt.float32, tag="ps_s0", bufs=2, name="psp")
                        for cc in range(2):
                            nc.tensor.matmul(psp[:], Wt["proj_wt"][cc][:, 128 * m:128 * m + 128],
                                             ao[cc][:, 512 * n_:512 * n_ + 512],
                                             start=(cc == 0), stop=(cc == 1))
                        nc.vector.scalar_tensor_tensor(
                            yf32[m][:, 512 * n_:512 * n_ + 512], psp[:], Wt["proj_b"][m][:],
                            yf32[m][:, 512 * n_:512 * n_ + 512], op0=ALU.add, op1=ALU.add)
                    nc.vector.tensor_copy(ym[:], yf32[m][:])
                    ymid.append(ym)

                # mlp
                hbf = []
                for mt in range(4):
                    hb = sp.tile([128, 1024], dt.bfloat16, tag="hbf", bufs=4, name="hbf")
                    for n_ in range(2):
                        ps1_ = ps_pool.tile([128, 512], dt.float32, tag="ps_s1", bufs=2, name="ps_mlp1")
                        for cc in range(2):
                            nc.tensor.matmul(ps1_[:], Wt["mlp1_wt"][cc][:, 128 * mt:128 * mt + 128],
                                             ymid[cc][:, 512 * n_:512 * n_ + 512],
                                             start=(cc == 0), stop=(cc == 1))
                        nc.scalar.activation(hb[:, 512 * n_:512 * n_ + 512], ps1_[:],
                                             AF.Silu, bias=Wt["mlp1_b"][mt][:])
                    hbf.append(hb)
                for m in range(2):
                    for n_ in range(2):
                        ps2_ = ps_pool.tile([128, 512], dt.float32, tag="ps_s0", bufs=2, name="ps_mlp2")
                        for cc in range(4):
                            nc.tensor.matmul(ps2_[:], Wt["mlp2_wt"][cc][:, 128 * m:128 * m + 128],
                                             hbf[cc][:, 512 * n_:512 * n_ + 512],
                                             start=(cc == 0), stop=(cc == 3))
                        nc.vector.scalar_tensor_tensor(
                            yf32[m][:, 512 * n_:512 * n_ + 512], ps2_[:], Wt["mlp2_b"][m][:],
                            yf32[m][:, 512 * n_:512 * n_ + 512], op0=ALU.add, op1=ALU.add)
                    nc.vector.tensor_copy(ybf[t + 1][m][:], yf32[m][:])

            # cv2 + delta out (final x-residual added host-side)
            ycat = [ybf[0][0], ybf[0][1], ybf[min(2, NBB)][0], ybf[min(2, NBB)][1], ybf[NBB][0], ybf[NBB][1]]
            for m in range(4):
                for n_ in range(2):
                    psf = ps_pool.tile([128, 512], dt.float32, tag="ps_s1", bufs=2, name="ps_cv2")
                    for cc in range(6):
                        nc.tensor.matmul(psf[:], cv2_wt[cc][:, 128 * m:128 * m + 128],
                                         ycat[cc][:, 512 * n_:512 * n_ + 512],
                                         start=(cc == 0), stop=(cc == 5))
                    os_ = sp.tile([128, 512], dt.float32, tag="os", name="os_")
                    nc.scalar.activation(os_[:], psf[:], AF.Silu, bias=cv2_b[m][:])
                    dl = sp.tile([128, 512], dt.float32, tag="dl", name="dl")
                    nc.vector.tensor_scalar(dl[:], os_[:], gamma[m][:], None, op0=ALU.mult)
                    nc.sync.dma_start(out_ext[128 * m:128 * m + 128, 512 * n_:512 * n_ + 512],
                                      dl[:])

    nc.compile()
    return nc


def _prep_inputs(x, params):
    xf = np.asarray(x, np.float32).reshape(B, C1, N)
    f32 = lambda a: np.ascontiguousarray(np.asarray(a, np.float32))
    bfc = lambda a: np.ascontiguousarray(np.asarray(a, np.float32).astype(bf16))
    col = lambda a: f32(a).reshape(-1, 1)

    shared = {
        "cv1_wt": bfc(np.asarray(params["cv1_w"]).T), "cv1_b": col(params["cv1_b"]),
        "cv2_wt": bfc(np.asarray(params["cv2_w"]).T), "cv2_b": col(params["cv2_b"]),
        "gamma": col(params["gamma"]),
    }
    blocks = [bp for stack in params["blocks"] for bp in stack]
    for t, bp in enumerate(blocks):
        qkw = np.asarray(bp["qk_w"], np.float32)
        shared[f"b{t}_qk_wt"] = bfc(qkw.T)
        qkb = np.asarray(bp["qk_b"], np.float32)
        qb = np.zeros((CH,), np.float32); kb = np.zeros((CH,), np.float32)
        for pg in range(2):
            for half in range(2):
                cg = 2 * pg + half
                qb[128 * pg + 64 * half:128 * pg + 64 * half + 64] = \
                    qkb[128 * cg:128 * cg + 64]
                kb[128 * pg + 64 * half:128 * pg + 64 * half + 64] = \
                    qkb[128 * cg + 64:128 * cg + 128]
        shared[f"b{t}_qb"] = qb.reshape(-1, 1); shared[f"b{t}_kb"] = kb.reshape(-1, 1)
        shared[f"b{t}_v_wt"] = bfc(np.asarray(bp["v_w"]).T)
        vb = np.asarray(bp["v_b"], np.float32)
        shared[f"b{t}_v_b"] = vb.reshape(-1, 1)
        vbd = np.zeros((8, 128), np.float32)
        for g in range(4):
            for hh in range(2):
                for h2 in range(4):
                    for uu in range(8):
                        for qh in range(4):
                            vbd[2 * g + hh, 32 * h2 + 4 * uu + qh] = \
                                vb[64 * g + 8 * (4 * hh + h2) + uu]
        shared[f"b{t}_vb_d"] = vbd.reshape(-1, 1)
        shared[f"b{t}_pe_w"] = f32(np.asarray(bp["pe_w"]).reshape(CH, 25))
        shared[f"b{t}_pe_b"] = col(bp["pe_b"])
        shared[f"b{t}_proj_wt"] = bfc(np.asarray(bp["proj_w"]).T)
        shared[f"b{t}_proj_b"] = col(bp["proj_b"])
        shared[f"b{t}_mlp1_wt"] = bfc(np.asarray(bp["mlp1_w"]).T)
        shared[f"b{t}_mlp1_b"] = col(bp["mlp1_b"])
        shared[f"b{t}_mlp2_wt"] = bfc(np.asarray(bp["mlp2_w"]).T)
        shared[f"b{t}_mlp2_b"] = col(bp["mlp2_b"])

    in_maps = []
    for core in range(NC):
        b, s = core // 4, core % 4
        cols = (np.arange(4)[:, None] * 1024 + 256 * s + np.arange(256)[None, :]).reshape(-1)
        m = dict(shared)
        m["x_bf"] = np.ascontiguousarray(xf[b][:, cols].astype(bf16))
        in_maps.append(m)
    return in_maps, xf


def _run(x, params, trace=False):
    if "nc" not in _cache:
        _cache["nc"] = _build()
    nc = _cache["nc"]
    in_maps, xf = _prep_inputs(x, params)
    res = run_bass_kernel_spmd(nc, in_maps, core_ids=list(range(NC)), trace=trace)
    out = np.empty((B, C2, N), np.float32)
    for core in range(NC):
        b, s = core // 4, core % 4
        cols = (np.arange(4)[:, None] * 1024 + 256 * s + np.arange(256)[None, :]).reshape(-1)
        out[b][:, cols] = xf[b][:, cols] + res.results[core]["out"]
    return out.reshape(B, C2, H, W), res


def kernel(x, params):
    out, _ = _run(x, params, trace=False)
    return out
